# revision 10
# baseline (speedup 1.0000x reference)
"""Sparse regional cross-attention on 8 Trainium2 NeuronCores.

Reference computation (B=1, S=56320, H=8, D=64, P=128, R=2):
  - per-region binary masks over the latent sequence (trilinear-resized video
    masks, thresholded) select which KV segments each query may attend to
  - regional pass: softmax over the union of allowed segments (global prompt
    + R regional prompts, 128 keys each)
  - base pass: plain softmax attention over the global prompt
  - out = 0.5 * regional + 0.5 * base

Kernel strategy:
  - sequence-parallel across 8 cores (7040 queries/core, all heads); no
    collectives needed (attention is cross-only)
  - per (head, 512-query tile), bf16 matmuls, queries on the free dim:
      scoresT [128 keys, W] = [kT*scale ; sel].T @ [qT ; b1 ; b2]   (K=66)
    region masks enter segments 1/2 as -30 additive bias rows, so
      E_r = exp(scoresT_r)  (ACT, bf16 out)  is already masked for r=1,2
      T0  [128 q, 65] = E0_chunk.T @ [V0 | 2.0]          (unmasked, base+seg0)
      T12 [128 q, 65] = E1.T @ [V1|2] + E2.T @ [V2|2]    (PSUM-accumulated)
    (lhsT=E gives query-major outputs natively - no transposes anywhere;
     col 64 is 2*sum(exp) so the 0.5 blend folds into the reciprocals)
  - merge per query (a0 = "covered by no region", per-partition scalars):
      wd = a0*T0d + T12d ; rr = 1/wd ; rb = 1/T0d ; c0 = a0*rr + rb
      out = c0*T0n + rr*T12n
  - q is pre-transposed/biased on host to bf16 [H, 66, S] so all DMA loads
    are contiguous; masks preprocessed on host (exact replica of the
    reference's jax trilinear resize).
"""

import sys

for _p in ("/opt/trn_rl_repo",):
    if _p not in sys.path:
        sys.path.insert(0, _p)

import numpy as np
import ml_dtypes

# ---------------------------------------------------------------- constants
B, S, H, D, P, R = 1, 56320, 8, 64, 128, 2
N_CORES = 8
SSH = S // N_CORES          # 7040 queries per core
W_TILE = 512                # queries per tile
N_TILES = (SSH + W_TILE - 1) // W_TILE   # 14 (13 full + 1x384)
LAT_T, LAT_H, LAT_W = 16, 44, 80
SCALE = D ** -0.5
NEG_BIAS = -30.0

_COMPILED = {}


# ------------------------------------------------------------ mask pipeline
def _resize_trilinear_np(m, tgt_shape):
    """numpy replica of jax.image.resize(..., 'trilinear', antialias=False)."""
    Bn, C, T, Hh, Ww = m.shape
    _, _, tT, tH, tW = tgt_shape
    out = m.astype(np.float32)

    def lin_weights(n_in, n_out):
        j = np.arange(n_out, dtype=np.float64)
        x = (j + 0.5) * (n_in / n_out) - 0.5
        lo = np.floor(x).astype(np.int64)
        frac = (x - lo).astype(np.float32)
        lo0 = np.clip(lo, 0, n_in - 1)
        lo1 = np.clip(lo + 1, 0, n_in - 1)
        Wm = np.zeros((n_out, n_in), np.float32)
        Wm[np.arange(n_out), lo0] += 1.0 - frac
        Wm[np.arange(n_out), lo1] += frac
        return Wm

    out = np.einsum('oi,bcihw->bcohw', lin_weights(T, tT), out)
    out = np.einsum('oi,bctiw->bctow', lin_weights(Hh, tH), out)
    out = np.einsum('oi,bcthi->bctho', lin_weights(Ww, tW), out)
    return out.astype(np.float32)


def _preprocess_mask_np(mask):
    m = np.transpose(mask, (3, 0, 1, 2))[:, None]  # [B,1,T,H,W]
    Bn = m.shape[0]
    T = m.shape[2]
    tgt = (Bn, 1, 1, LAT_H, LAT_W)
    pieces = [_resize_trilinear_np(m[:, :, :1], tgt)]
    for wi in range(1, T, 8):
        pieces.append(_resize_trilinear_np(m[:, :, wi:wi + 8], tgt))
    mm = np.concatenate(pieces, axis=2)[:, 0]
    return (mm > 0.5).astype(np.float32).reshape(Bn, -1)


def _preprocess_masks(region_masks):
    """region_masks [R, T, MH, MW, B] -> a0, a1, a2 each [S] float32 {0,1}."""
    try:
        import jax
        import jax.numpy as jnp

        cpu = jax.devices('cpu')[0]
        with jax.default_device(cpu):
            def one(mask):
                m = jnp.transpose(jnp.asarray(mask), (3, 0, 1, 2))[:, None]
                Bn, _, T, _, _ = m.shape
                tgt = (Bn, 1, 1, LAT_H, LAT_W)
                pieces = [jax.image.resize(m[:, :, :1], tgt, 'trilinear',
                                           antialias=False)]
                for wi in range(1, T, 8):
                    pieces.append(jax.image.resize(m[:, :, wi:wi + 8], tgt,
                                                   'trilinear',
                                                   antialias=False))
                mm = jnp.concatenate(pieces, axis=2)[:, 0]
                return (mm > 0.5).astype(jnp.float32).reshape(Bn, -1)

            masks = np.stack([np.asarray(one(region_masks[i]))
                              for i in range(region_masks.shape[0])], axis=0)
    except Exception:
        masks = np.stack([_preprocess_mask_np(region_masks[i])
                          for i in range(region_masks.shape[0])], axis=0)
    a1 = masks[0, 0]
    a2 = masks[1, 0]
    a0 = ((masks[0, 0] + masks[1, 0]) == 0).astype(np.float32)
    return a0, a1, a2


# ------------------------------------------------------------- bass kernel
def _build_kernel():
    import concourse.bass as bass
    import concourse.tile as tile
    from concourse import bacc, mybir

    f32 = mybir.dt.float32
    bf16 = mybir.dt.bfloat16
    Exp = mybir.ActivationFunctionType.Exp
    mult = mybir.AluOpType.mult
    add = mybir.AluOpType.add

    nc = bacc.Bacc("TRN2", target_bir_lowering=False, debug=False,
                   num_devices=N_CORES)

    qt_d = nc.dram_tensor("qt", [H, 66, SSH], bf16, kind="ExternalInput").ap()
    kt_d = nc.dram_tensor("kt", [3, H, 66, P], bf16, kind="ExternalInput").ap()
    vp_d = nc.dram_tensor("vp", [3, H, P, 65], bf16, kind="ExternalInput").ap()
    am_d = nc.dram_tensor("am", [N_TILES, 128, 4], f32,
                          kind="ExternalInput").ap()
    out_d = nc.dram_tensor("out", [SSH, H * D], f32,
                           kind="ExternalOutput").ap()

    with tile.TileContext(nc) as tc:
        with (
            tc.tile_pool(name="singles", bufs=1) as singles,
            tc.tile_pool(name="qt", bufs=2) as qt_pool,
            tc.tile_pool(name="am", bufs=2) as am_pool,
            tc.tile_pool(name="scores", bufs=2, space="PSUM") as sc_pool,
            tc.tile_pool(name="epool", bufs=8) as e_pool,
            tc.tile_pool(name="t0pool", bufs=1, space="PSUM") as t0_pool,
            tc.tile_pool(name="t12pool", bufs=1, space="PSUM") as t12_pool,
            tc.tile_pool(name="small", bufs=16) as sm_pool,
            tc.tile_pool(name="tmp", bufs=8) as tmp_pool,
            tc.tile_pool(name="slab", bufs=2) as slab_pool,
        ):
            kt_sb = singles.tile([66, 3, H, P], bf16)
            nc.sync.dma_start(out=kt_sb, in_=kt_d.rearrange("s h d p -> d s h p"))
            vp_sb = singles.tile([128, 3, H, 65], bf16)
            nc.sync.dma_start(out=vp_sb, in_=vp_d.rearrange("s h p c -> p s h c"))

            for t in range(N_TILES):
                s0 = t * W_TILE
                Wq = min(W_TILE, SSH - s0)
                nch = Wq // 128

                qt_t = qt_pool.tile([66, H, W_TILE], bf16)
                nc.sync.dma_start(
                    out=qt_t[:, :, :Wq],
                    in_=qt_d.rearrange("h d s -> d h s")[:, :, s0:s0 + Wq])
                am_t = am_pool.tile([128, 4], f32)
                nc.sync.dma_start(out=am_t, in_=am_d[t])

                slab = slab_pool.tile([128, 4, H * D], f32)

                for h in range(H):
                    sc = sc_pool.tile([128, 3, W_TILE], f32, tag="scores")
                    for r in range(3):
                        nc.tensor.matmul(
                            sc[:, r, :Wq],
                            lhsT=kt_sb[:, r, h, :],
                            rhs=qt_t[:, h, :Wq],
                            start=True, stop=True)
                    eall = e_pool.tile([128, 3, W_TILE], bf16, tag="e")
                    if Wq == W_TILE:
                        nc.scalar.activation(eall, sc, Exp)
                    else:
                        nc.scalar.activation(eall[:, :, :Wq], sc[:, :, :Wq],
                                             Exp)
                    es = [eall[:, 0, :], eall[:, 1, :], eall[:, 2, :]]

                    T0 = t0_pool.tile([128, 4, 65], f32, tag="T0")
                    T12 = t12_pool.tile([128, 4, 65], f32, tag="T12")
                    for c in range(nch):
                        cs = slice(c * 128, (c + 1) * 128)
                        nc.tensor.matmul(T0[:, c, :], lhsT=es[0][:, cs],
                                         rhs=vp_sb[:, 0, h, :],
                                         start=True, stop=True)
                        nc.tensor.matmul(T12[:, c, :], lhsT=es[1][:, cs],
                                         rhs=vp_sb[:, 1, h, :],
                                         start=True, stop=False)
                        nc.tensor.matmul(T12[:, c, :], lhsT=es[2][:, cs],
                                         rhs=vp_sb[:, 2, h, :],
                                         start=False, stop=True)

                    a0 = am_t[:, 0:nch]
                    # out = rb*T0n + rr*(a0*T0n + T12n)
                    # wd = a0*T0d + T12d ; rr = 1/wd ; rb = 1/T0d
                    # T-release path is all-DVE; GpSimd only touches SBUF.
                    m0 = sm_pool.tile([128, 4], f32, tag="sm")
                    nc.vector.tensor_mul(m0[:, :nch], a0, T0[:, :nch, 64])
                    wd = sm_pool.tile([128, 4], f32, tag="sm")
                    nc.vector.tensor_add(wd[:, :nch], m0[:, :nch],
                                         T12[:, :nch, 64])
                    rb = sm_pool.tile([128, 4], f32, tag="sm")
                    nc.vector.reciprocal(rb[:, :nch], T0[:, :nch, 64])
                    rr = sm_pool.tile([128, 4], f32, tag="sm")
                    nc.vector.reciprocal(rr[:, :nch], wd[:, :nch])

                    tu = tmp_pool.tile([128, 4, 64], f32, tag="tu")
                    nc.vector.tensor_mul(
                        tu[:, :nch, :], T0[:, :nch, 0:64],
                        a0[:, :, None].broadcast_to([128, nch, 64]))
                    uu = tmp_pool.tile([128, 4, 64], f32, tag="uu")
                    nc.vector.tensor_add(uu[:, :nch, :], tu[:, :nch, :],
                                         T12[:, :nch, 0:64])
                    ww = tmp_pool.tile([128, 4, 64], f32, tag="ww")
                    nc.vector.tensor_mul(
                        ww[:, :nch, :], T0[:, :nch, 0:64],
                        rb[:, :nch, None].broadcast_to([128, nch, 64]))

                    g1 = tmp_pool.tile([128, 4, 64], f32, tag="g1")
                    nc.gpsimd.tensor_mul(
                        g1[:, :nch, :], uu[:, :nch, :],
                        rr[:, :nch, None].broadcast_to([128, nch, 64]))
                    nc.gpsimd.tensor_add(
                        slab[:, :nch, h * 64:(h + 1) * 64],
                        g1[:, :nch, :], ww[:, :nch, :])

                nc.sync.dma_start(
                    out=out_d[s0:s0 + Wq, :].rearrange("(c p) f -> p c f", p=128),
                    in_=slab[:, :nch, :])

    nc.compile()
    return nc


def _get_compiled():
    if "nc" not in _COMPILED:
        _COMPILED["nc"] = _build_kernel()
    return _COMPILED["nc"]


# ---------------------------------------------------------------- frontend
def _prepare_in_maps(q, k, v, regional_k, regional_v, region_masks):
    bf = ml_dtypes.bfloat16
    q = np.asarray(q, dtype=np.float32)
    k = np.asarray(k, dtype=np.float32)
    v = np.asarray(v, dtype=np.float32)
    regional_k = np.asarray(regional_k, dtype=np.float32)
    regional_v = np.asarray(regional_v, dtype=np.float32)
    region_masks = np.asarray(region_masks, dtype=np.float32)

    a0, a1, a2 = _preprocess_masks(region_masks)  # [S] each
    b1 = (NEG_BIAS * (1.0 - a1)).astype(bf)       # [S] 0 / -30
    b2 = (NEG_BIAS * (1.0 - a2)).astype(bf)

    # qT plus bias rows: [H, 66, S] bf16
    qt = np.empty((H, 66, S), dtype=bf)
    qt[:, :64, :] = q[0].transpose(1, 2, 0).astype(bf)
    qt[:, 64, :] = b1[None, :]
    qt[:, 65, :] = b2[None, :]

    # kT*scale plus selector rows: [3, H, 66, P] bf16
    k_segs = np.stack([k[0], regional_k[0, 0], regional_k[1, 0]], axis=0)
    kt = np.zeros((3, H, 66, P), dtype=np.float32)
    kt[:, :, :64, :] = k_segs.transpose(0, 2, 3, 1) * np.float32(SCALE)
    kt[1, :, 64, :] = 1.0
    kt[2, :, 65, :] = 1.0
    kt = kt.astype(bf)

    # V plus 2.0-column: [3, H, P, 65] bf16
    v_segs = np.stack([v[0], regional_v[0, 0], regional_v[1, 0]], axis=0)
    vp = np.empty((3, H, P, 65), dtype=np.float32)
    vp[..., :64] = v_segs.transpose(0, 2, 1, 3)
    vp[..., 64] = 2.0
    vp = vp.astype(bf)

    in_maps = []
    for core in range(N_CORES):
        lo = core * SSH
        am = np.zeros((N_TILES, 128, 4), np.float32)
        for t in range(N_TILES):
            s0 = t * W_TILE
            Wq = min(W_TILE, SSH - s0)
            nch = Wq // 128
            am[t, :, :nch] = a0[lo + s0: lo + s0 + Wq].reshape(nch, 128).T
        in_maps.append({
            "qt": np.ascontiguousarray(qt[:, :, lo:lo + SSH]),
            "kt": kt,
            "vp": vp,
            "am": am,
        })
    return in_maps


def kernel(q, k, v, regional_k, regional_v, region_masks):
    from concourse.bass_utils import run_bass_kernel_spmd

    nc = _get_compiled()
    in_maps = _prepare_in_maps(q, k, v, regional_k, regional_v, region_masks)
    res = run_bass_kernel_spmd(nc, in_maps, core_ids=list(range(N_CORES)))
    out = np.concatenate([res.results[i]["out"] for i in range(N_CORES)],
                         axis=0)
    return out.reshape(1, S, H * D).astype(np.float32)


# revision 11
# speedup vs baseline: 1.5903x; 1.5903x over previous
"""Sparse regional cross-attention on 8 Trainium2 NeuronCores.

Reference computation (B=1, S=56320, H=8, D=64, P=128, R=2):
  - per-region binary masks over the latent sequence select which KV segments
    each query may attend to (global prompt + R regional prompts, 128 keys
    each); regional pass = softmax over the union of allowed segments
  - base pass: plain softmax attention over the global prompt
  - out = 0.5 * regional + 0.5 * base

Kernel strategy:
  - sequence-parallel across 8 cores (7040 queries/core, all heads); no
    collectives (attention is cross-only, no q-q interaction)
  - queries are HOST-SORTED by mask category: queries covered by no region
    (a0=1, ~25%) only need segment 0 (regional pass == base pass there), so
    they are packed into leading "global" tiles that skip segments 1/2
    entirely; output rows are un-permuted on host after the gather
  - per (head, 512-query tile), bf16 matmuls, queries on the free dim:
      scoresT [128 keys, W] = [kT*scale ; sel].T @ [qT ; b1 ; b2]   (K=66)
    region masks enter segments 1/2 as -30 additive bias rows, so
      E_r = exp(scoresT_r)  (ACT, bf16 out)  is already masked for r=1,2
      T0  [128 q, 65] = E0_chunk.T @ [V0 | 2.0]          (unmasked, base+seg0)
      T12 [128 q, 65] = E1.T @ [V1|2] + E2.T @ [V2|2]    (PSUM-accumulated)
    (lhsT=E gives query-major outputs natively - no transposes anywhere;
     col 64 is 2*sum(exp) so the 0.5 blend folds into the reciprocals)
  - merge per query (all per-partition scalar ops):
      regional tiles: wd = a0*T0d + T12d ; rr = 1/wd ; rb = 1/T0d
                      out = (a0*rr + rb)*T0n + rr*T12n
      global tiles:   out = T0n / (0.5*T0d)
"""

import sys

for _p in ("/opt/trn_rl_repo",):
    if _p not in sys.path:
        sys.path.insert(0, _p)

import numpy as np
import ml_dtypes

# ---------------------------------------------------------------- constants
B, S, H, D, P, R = 1, 56320, 8, 64, 128, 2
N_CORES = 8
SSH = S // N_CORES          # 7040 queries per core
W_TILE = 512                # queries per tile
N_TILES = (SSH + W_TILE - 1) // W_TILE   # 14 (13 full + 1x384)
LAT_T, LAT_H, LAT_W = 16, 44, 80
SCALE = D ** -0.5
NEG_BIAS = -30.0

_COMPILED = {}


# ------------------------------------------------------------ mask pipeline
def _resize_trilinear_np(m, tgt_shape):
    """numpy replica of jax.image.resize(..., 'trilinear', antialias=False)."""
    Bn, C, T, Hh, Ww = m.shape
    _, _, tT, tH, tW = tgt_shape
    out = m.astype(np.float32)

    def lin_weights(n_in, n_out):
        j = np.arange(n_out, dtype=np.float64)
        x = (j + 0.5) * (n_in / n_out) - 0.5
        lo = np.floor(x).astype(np.int64)
        frac = (x - lo).astype(np.float32)
        lo0 = np.clip(lo, 0, n_in - 1)
        lo1 = np.clip(lo + 1, 0, n_in - 1)
        Wm = np.zeros((n_out, n_in), np.float32)
        Wm[np.arange(n_out), lo0] += 1.0 - frac
        Wm[np.arange(n_out), lo1] += frac
        return Wm

    out = np.einsum('oi,bcihw->bcohw', lin_weights(T, tT), out)
    out = np.einsum('oi,bctiw->bctow', lin_weights(Hh, tH), out)
    out = np.einsum('oi,bcthi->bctho', lin_weights(Ww, tW), out)
    return out.astype(np.float32)


def _preprocess_mask_np(mask):
    m = np.transpose(mask, (3, 0, 1, 2))[:, None]  # [B,1,T,H,W]
    Bn = m.shape[0]
    T = m.shape[2]
    tgt = (Bn, 1, 1, LAT_H, LAT_W)
    pieces = [_resize_trilinear_np(m[:, :, :1], tgt)]
    for wi in range(1, T, 8):
        pieces.append(_resize_trilinear_np(m[:, :, wi:wi + 8], tgt))
    mm = np.concatenate(pieces, axis=2)[:, 0]
    return (mm > 0.5).astype(np.float32).reshape(Bn, -1)


def _preprocess_masks(region_masks):
    """region_masks [R, T, MH, MW, B] -> a0, a1, a2 each [S] float32 {0,1}."""
    try:
        import jax
        import jax.numpy as jnp

        cpu = jax.devices('cpu')[0]
        with jax.default_device(cpu):
            def one(mask):
                m = jnp.transpose(jnp.asarray(mask), (3, 0, 1, 2))[:, None]
                Bn, _, T, _, _ = m.shape
                tgt = (Bn, 1, 1, LAT_H, LAT_W)
                pieces = [jax.image.resize(m[:, :, :1], tgt, 'trilinear',
                                           antialias=False)]
                for wi in range(1, T, 8):
                    pieces.append(jax.image.resize(m[:, :, wi:wi + 8], tgt,
                                                   'trilinear',
                                                   antialias=False))
                mm = jnp.concatenate(pieces, axis=2)[:, 0]
                return (mm > 0.5).astype(jnp.float32).reshape(Bn, -1)

            masks = np.stack([np.asarray(one(region_masks[i]))
                              for i in range(region_masks.shape[0])], axis=0)
    except Exception:
        masks = np.stack([_preprocess_mask_np(region_masks[i])
                          for i in range(region_masks.shape[0])], axis=0)
    a1 = masks[0, 0]
    a2 = masks[1, 0]
    a0 = ((masks[0, 0] + masks[1, 0]) == 0).astype(np.float32)
    return a0, a1, a2


# ------------------------------------------------------------- bass kernel
def _build_kernel(gt):
    """gt = number of leading pure-global (segment-0-only) tiles per core."""
    import concourse.bass as bass
    import concourse.tile as tile
    from concourse import bacc, mybir

    f32 = mybir.dt.float32
    bf16 = mybir.dt.bfloat16
    Exp = mybir.ActivationFunctionType.Exp
    mult = mybir.AluOpType.mult
    add = mybir.AluOpType.add

    nc = bacc.Bacc("TRN2", target_bir_lowering=False, debug=False,
                   num_devices=N_CORES)

    qt_d = nc.dram_tensor("qt", [H, 66, SSH], bf16, kind="ExternalInput").ap()
    kt_d = nc.dram_tensor("kt", [3, H, 66, P], bf16, kind="ExternalInput").ap()
    vp_d = nc.dram_tensor("vp", [3, H, P, 65], bf16, kind="ExternalInput").ap()
    am_d = nc.dram_tensor("am", [N_TILES, 128, 4], f32,
                          kind="ExternalInput").ap()
    out_d = nc.dram_tensor("out", [SSH, H * D], f32,
                           kind="ExternalOutput").ap()

    with tile.TileContext(nc) as tc:
        with (
            tc.tile_pool(name="singles", bufs=1) as singles,
            tc.tile_pool(name="qt", bufs=2) as qt_pool,
            tc.tile_pool(name="am", bufs=2) as am_pool,
            tc.tile_pool(name="scores", bufs=4, space="PSUM") as sc_pool,
            tc.tile_pool(name="epool", bufs=8) as e_pool,
            tc.tile_pool(name="t0pool", bufs=2, space="PSUM") as t0_pool,
            tc.tile_pool(name="t12pool", bufs=2, space="PSUM") as t12_pool,
            tc.tile_pool(name="small", bufs=16) as sm_pool,
            tc.tile_pool(name="tmp", bufs=8) as tmp_pool,
            tc.tile_pool(name="slab", bufs=2) as slab_pool,
        ):
            kt_sb = singles.tile([66, 3, H, P], bf16)
            nc.sync.dma_start(out=kt_sb, in_=kt_d.rearrange("s h d p -> d s h p"))
            vp_sb = singles.tile([128, 3, H, 65], bf16)
            nc.sync.dma_start(out=vp_sb, in_=vp_d.rearrange("s h p c -> p s h c"))

            for t in range(N_TILES):
                s0 = t * W_TILE
                Wq = min(W_TILE, SSH - s0)
                nch = Wq // 128
                is_global = t < gt

                qt_t = qt_pool.tile([66, H, W_TILE], bf16)
                nc.sync.dma_start(
                    out=qt_t[:, :, :Wq],
                    in_=qt_d.rearrange("h d s -> d h s")[:, :, s0:s0 + Wq])
                if not is_global:
                    am_t = am_pool.tile([128, 4], f32)
                    nc.sync.dma_start(out=am_t, in_=am_d[t])

                slab = slab_pool.tile([128, 4, H * D], f32)

                for h in range(H):
                    if is_global:
                        # segment-0 only: regional == base here
                        sc = sc_pool.tile([128, W_TILE], f32, tag="scores")
                        nc.tensor.matmul(
                            sc[:, :Wq], lhsT=kt_sb[:, 0, h, :],
                            rhs=qt_t[:, h, :Wq], start=True, stop=True)
                        e = e_pool.tile([128, W_TILE], bf16, tag="e")
                        nc.scalar.activation(e[:, :Wq], sc[:, :Wq], Exp)
                        T0 = t0_pool.tile([128, 4, 65], f32, tag="T0")
                        for c in range(nch):
                            cs = slice(c * 128, (c + 1) * 128)
                            nc.tensor.matmul(T0[:, c, :], lhsT=e[:, cs],
                                             rhs=vp_sb[:, 0, h, :],
                                             start=True, stop=True)
                        # out = T0n / (0.5 * T0d)
                        hd = sm_pool.tile([128, 4], f32, tag="sm")
                        nc.vector.tensor_scalar_mul(hd[:, :nch],
                                                    T0[:, :nch, 64], 0.5)
                        rg = sm_pool.tile([128, 4], f32, tag="sm")
                        nc.vector.reciprocal(rg[:, :nch], hd[:, :nch])
                        nc.vector.tensor_mul(
                            slab[:, :nch, h * 64:(h + 1) * 64],
                            T0[:, :nch, 0:64],
                            rg[:, :nch, None].broadcast_to([128, nch, 64]))
                        continue

                    es = []
                    for r in range(3):
                        sc = sc_pool.tile([128, W_TILE], f32, tag="scores")
                        nc.tensor.matmul(
                            sc[:, :Wq],
                            lhsT=kt_sb[:, r, h, :],
                            rhs=qt_t[:, h, :Wq],
                            start=True, stop=True)
                        e = e_pool.tile([128, W_TILE], bf16, tag="e")
                        nc.scalar.activation(e[:, :Wq], sc[:, :Wq], Exp)
                        es.append(e)

                    T0 = t0_pool.tile([128, 4, 65], f32, tag="T0")
                    T12 = t12_pool.tile([128, 4, 65], f32, tag="T12")
                    for c in range(nch):
                        cs = slice(c * 128, (c + 1) * 128)
                        nc.tensor.matmul(T0[:, c, :], lhsT=es[0][:, cs],
                                         rhs=vp_sb[:, 0, h, :],
                                         start=True, stop=True)
                        nc.tensor.matmul(T12[:, c, :], lhsT=es[1][:, cs],
                                         rhs=vp_sb[:, 1, h, :],
                                         start=True, stop=False)
                        nc.tensor.matmul(T12[:, c, :], lhsT=es[2][:, cs],
                                         rhs=vp_sb[:, 2, h, :],
                                         start=False, stop=True)

                    a0 = am_t[:, 0:nch]
                    # wd = a0*T0d + T12d ; rr = 1/wd ; rb = 1/T0d
                    # out = (a0*rr + rb)*T0n + rr*T12n
                    m0 = sm_pool.tile([128, 4], f32, tag="sm")
                    nc.vector.tensor_mul(m0[:, :nch], a0, T0[:, :nch, 64])
                    wd = sm_pool.tile([128, 4], f32, tag="sm")
                    nc.vector.tensor_add(wd[:, :nch], m0[:, :nch],
                                         T12[:, :nch, 64])
                    rb = sm_pool.tile([128, 4], f32, tag="sm")
                    nc.vector.reciprocal(rb[:, :nch], T0[:, :nch, 64])
                    rr = sm_pool.tile([128, 4], f32, tag="sm")
                    nc.vector.reciprocal(rr[:, :nch], wd[:, :nch])
                    c0a = sm_pool.tile([128, 4], f32, tag="sm")
                    nc.gpsimd.tensor_mul(c0a[:, :nch], a0, rr[:, :nch])
                    c0b = sm_pool.tile([128, 4], f32, tag="sm")
                    nc.gpsimd.tensor_add(c0b[:, :nch], c0a[:, :nch],
                                         rb[:, :nch])

                    t1 = tmp_pool.tile([128, 4, 64], f32, tag="t1")
                    nc.vector.tensor_mul(
                        t1[:, :nch, :], T0[:, :nch, 0:64],
                        c0b[:, :nch, None].broadcast_to([128, nch, 64]))
                    t2 = tmp_pool.tile([128, 4, 64], f32, tag="t2")
                    nc.vector.tensor_mul(
                        t2[:, :nch, :], T12[:, :nch, 0:64],
                        rr[:, :nch, None].broadcast_to([128, nch, 64]))
                    nc.gpsimd.tensor_add(
                        slab[:, :nch, h * 64:(h + 1) * 64],
                        t1[:, :nch, :], t2[:, :nch, :])

                nc.sync.dma_start(
                    out=out_d[s0:s0 + Wq, :].rearrange("(c p) f -> p c f", p=128),
                    in_=slab[:, :nch, :])

    nc.compile()
    return nc


def _get_compiled(gt):
    if gt not in _COMPILED:
        _COMPILED[gt] = _build_kernel(gt)
    return _COMPILED[gt]


# ---------------------------------------------------------------- frontend
def _prepare(q, k, v, regional_k, regional_v, region_masks):
    bf = ml_dtypes.bfloat16
    q = np.asarray(q, dtype=np.float32)
    k = np.asarray(k, dtype=np.float32)
    v = np.asarray(v, dtype=np.float32)
    regional_k = np.asarray(regional_k, dtype=np.float32)
    regional_v = np.asarray(regional_v, dtype=np.float32)
    region_masks = np.asarray(region_masks, dtype=np.float32)

    a0, a1, a2 = _preprocess_masks(region_masks)  # [S] each

    # sort queries: per core, `gt` leading tiles hold only a0==1 queries
    idx_glob = np.nonzero(a0 == 1.0)[0]
    idx_rest = np.nonzero(a0 == 0.0)[0]
    G = len(idx_glob)
    gt = min(G // (N_CORES * W_TILE), N_TILES - 1)
    n_used = gt * W_TILE            # global queries per core
    used = idx_glob[:n_used * N_CORES]
    leftover = np.concatenate([idx_glob[n_used * N_CORES:], idx_rest])
    n_left = SSH - n_used           # leftover queries per core
    perm = np.empty(S, dtype=np.int64)
    for c in range(N_CORES):
        lo = c * SSH
        perm[lo:lo + n_used] = used[c * n_used:(c + 1) * n_used]
        perm[lo + n_used:lo + SSH] = leftover[c * n_left:(c + 1) * n_left]

    a0p = a0[perm]
    b1 = (NEG_BIAS * (1.0 - a1[perm])).astype(bf)
    b2 = (NEG_BIAS * (1.0 - a2[perm])).astype(bf)

    # qT plus bias rows: [H, 66, S] bf16, query-permuted
    qt = np.empty((H, 66, S), dtype=bf)
    qt[:, :64, :] = q[0].transpose(1, 2, 0)[:, :, perm].astype(bf)
    qt[:, 64, :] = b1[None, :]
    qt[:, 65, :] = b2[None, :]

    # kT*scale plus selector rows: [3, H, 66, P] bf16
    k_segs = np.stack([k[0], regional_k[0, 0], regional_k[1, 0]], axis=0)
    kt = np.zeros((3, H, 66, P), dtype=np.float32)
    kt[:, :, :64, :] = k_segs.transpose(0, 2, 3, 1) * np.float32(SCALE)
    kt[1, :, 64, :] = 1.0
    kt[2, :, 65, :] = 1.0
    kt = kt.astype(bf)

    # V plus 2.0-column: [3, H, P, 65] bf16
    v_segs = np.stack([v[0], regional_v[0, 0], regional_v[1, 0]], axis=0)
    vp = np.empty((3, H, P, 65), dtype=np.float32)
    vp[..., :64] = v_segs.transpose(0, 2, 1, 3)
    vp[..., 64] = 2.0
    vp = vp.astype(bf)

    in_maps = []
    for core in range(N_CORES):
        lo = core * SSH
        am = np.zeros((N_TILES, 128, 4), np.float32)
        for t in range(gt, N_TILES):
            s0 = t * W_TILE
            Wq = min(W_TILE, SSH - s0)
            nch = Wq // 128
            am[t, :, :nch] = a0p[lo + s0: lo + s0 + Wq].reshape(nch, 128).T
        in_maps.append({
            "qt": np.ascontiguousarray(qt[:, :, lo:lo + SSH]),
            "kt": kt,
            "vp": vp,
            "am": am,
        })
    return in_maps, perm, gt


def kernel(q, k, v, regional_k, regional_v, region_masks):
    from concourse.bass_utils import run_bass_kernel_spmd

    in_maps, perm, gt = _prepare(q, k, v, regional_k, regional_v,
                                 region_masks)
    nc = _get_compiled(gt)
    res = run_bass_kernel_spmd(nc, in_maps, core_ids=list(range(N_CORES)))
    out_sorted = np.concatenate(
        [res.results[i]["out"] for i in range(N_CORES)], axis=0)
    out = np.empty_like(out_sorted)
    out[perm] = out_sorted
    return out.reshape(1, S, H * D).astype(np.float32)


# revision 13
# speedup vs baseline: 1.8487x; 1.1625x over previous
"""Sparse regional cross-attention on 8 Trainium2 NeuronCores.

Reference computation (B=1, S=56320, H=8, D=64, P=128, R=2):
  - per-region binary masks over the latent sequence select which KV segments
    each query may attend to (global prompt + R regional prompts, 128 keys
    each); regional pass = softmax over the union of allowed segments
  - base pass: plain softmax attention over the global prompt
  - out = 0.5 * regional + 0.5 * base

Kernel strategy:
  - sequence-parallel across 8 cores (7040 queries/core, all heads); no
    collectives (attention is cross-only, no q-q interaction)
  - queries are HOST-SORTED by mask category: queries covered by no region
    (a0=1, ~25%) only need segment 0 (regional pass == base pass there), so
    they are packed into leading "global" tiles that skip segments 1/2
    entirely; output rows are un-permuted on host after the gather
  - per (head, 512-query tile), bf16 matmuls, queries on the free dim:
      scoresT [128 keys, W] = [kT*scale ; sel].T @ [qT ; b1 ; b2]   (K=66)
    region masks enter segments 1/2 as -30 additive bias rows, so
      E_r = exp(scoresT_r)  (ACT, bf16 out)  is already masked for r=1,2
      T0  [128 q, 65] = E0_chunk.T @ [V0 | 2.0]          (unmasked, base+seg0)
      T12 [128 q, 65] = E1.T @ [V1|2] + E2.T @ [V2|2]    (PSUM-accumulated)
    (lhsT=E gives query-major outputs natively - no transposes anywhere;
     col 64 is 2*sum(exp) so the 0.5 blend folds into the reciprocals)
  - merge per query (all per-partition scalar ops):
      regional tiles: wd = a0*T0d + T12d ; rr = 1/wd ; rb = 1/T0d
                      out = (a0*rr + rb)*T0n + rr*T12n
      global tiles:   out = T0n / (0.5*T0d)
"""

import sys

for _p in ("/opt/trn_rl_repo",):
    if _p not in sys.path:
        sys.path.insert(0, _p)

import numpy as np
import ml_dtypes

# ---------------------------------------------------------------- constants
B, S, H, D, P, R = 1, 56320, 8, 64, 128, 2
N_CORES = 8
SSH = S // N_CORES          # 7040 queries per core
W_TILE = 512                # queries per tile
N_TILES = (SSH + W_TILE - 1) // W_TILE   # 14 (13 full + 1x384)
LAT_T, LAT_H, LAT_W = 16, 44, 80
SCALE = D ** -0.5
NEG_BIAS = -30.0

_COMPILED = {}


# ------------------------------------------------------------ mask pipeline
def _resize_trilinear_np(m, tgt_shape):
    """numpy replica of jax.image.resize(..., 'trilinear', antialias=False)."""
    Bn, C, T, Hh, Ww = m.shape
    _, _, tT, tH, tW = tgt_shape
    out = m.astype(np.float32)

    def lin_weights(n_in, n_out):
        j = np.arange(n_out, dtype=np.float64)
        x = (j + 0.5) * (n_in / n_out) - 0.5
        lo = np.floor(x).astype(np.int64)
        frac = (x - lo).astype(np.float32)
        lo0 = np.clip(lo, 0, n_in - 1)
        lo1 = np.clip(lo + 1, 0, n_in - 1)
        Wm = np.zeros((n_out, n_in), np.float32)
        Wm[np.arange(n_out), lo0] += 1.0 - frac
        Wm[np.arange(n_out), lo1] += frac
        return Wm

    out = np.einsum('oi,bcihw->bcohw', lin_weights(T, tT), out)
    out = np.einsum('oi,bctiw->bctow', lin_weights(Hh, tH), out)
    out = np.einsum('oi,bcthi->bctho', lin_weights(Ww, tW), out)
    return out.astype(np.float32)


def _preprocess_mask_np(mask):
    m = np.transpose(mask, (3, 0, 1, 2))[:, None]  # [B,1,T,H,W]
    Bn = m.shape[0]
    T = m.shape[2]
    tgt = (Bn, 1, 1, LAT_H, LAT_W)
    pieces = [_resize_trilinear_np(m[:, :, :1], tgt)]
    for wi in range(1, T, 8):
        pieces.append(_resize_trilinear_np(m[:, :, wi:wi + 8], tgt))
    mm = np.concatenate(pieces, axis=2)[:, 0]
    return (mm > 0.5).astype(np.float32).reshape(Bn, -1)


def _preprocess_masks(region_masks):
    """region_masks [R, T, MH, MW, B] -> a0, a1, a2 each [S] float32 {0,1}."""
    try:
        import jax
        import jax.numpy as jnp

        cpu = jax.devices('cpu')[0]
        with jax.default_device(cpu):
            def one(mask):
                m = jnp.transpose(jnp.asarray(mask), (3, 0, 1, 2))[:, None]
                Bn, _, T, _, _ = m.shape
                tgt = (Bn, 1, 1, LAT_H, LAT_W)
                pieces = [jax.image.resize(m[:, :, :1], tgt, 'trilinear',
                                           antialias=False)]
                for wi in range(1, T, 8):
                    pieces.append(jax.image.resize(m[:, :, wi:wi + 8], tgt,
                                                   'trilinear',
                                                   antialias=False))
                mm = jnp.concatenate(pieces, axis=2)[:, 0]
                return (mm > 0.5).astype(jnp.float32).reshape(Bn, -1)

            masks = np.stack([np.asarray(one(region_masks[i]))
                              for i in range(region_masks.shape[0])], axis=0)
    except Exception:
        masks = np.stack([_preprocess_mask_np(region_masks[i])
                          for i in range(region_masks.shape[0])], axis=0)
    a1 = masks[0, 0]
    a2 = masks[1, 0]
    a0 = ((masks[0, 0] + masks[1, 0]) == 0).astype(np.float32)
    return a0, a1, a2


# ------------------------------------------------------------- bass kernel
def _build_kernel(cfg):
    """cfg = (t_g, t_r1, t_r2): leading tile counts per core for the
    global-only / region-1-only / region-2-only categories; the rest are
    general 3-segment tiles."""
    import concourse.bass as bass
    import concourse.tile as tile
    from concourse import bacc, mybir

    f32 = mybir.dt.float32
    bf16 = mybir.dt.bfloat16
    Exp = mybir.ActivationFunctionType.Exp
    mult = mybir.AluOpType.mult
    add = mybir.AluOpType.add

    nc = bacc.Bacc("TRN2", target_bir_lowering=False, debug=False,
                   num_devices=N_CORES)

    qt_d = nc.dram_tensor("qt", [H, 66, SSH], bf16, kind="ExternalInput").ap()
    kt_d = nc.dram_tensor("kt", [3, H, 66, P], bf16, kind="ExternalInput").ap()
    vp_d = nc.dram_tensor("vp", [3, H, P, 65], bf16, kind="ExternalInput").ap()
    am_d = nc.dram_tensor("am", [N_TILES, 128, 4], f32,
                          kind="ExternalInput").ap()
    out_d = nc.dram_tensor("out", [SSH, H * D], f32,
                           kind="ExternalOutput").ap()

    with tile.TileContext(nc) as tc:
        with (
            tc.tile_pool(name="singles", bufs=1) as singles,
            tc.tile_pool(name="qt", bufs=2) as qt_pool,
            tc.tile_pool(name="am", bufs=2) as am_pool,
            tc.tile_pool(name="scores", bufs=4, space="PSUM") as sc_pool,
            tc.tile_pool(name="epool", bufs=8) as e_pool,
            tc.tile_pool(name="t0pool", bufs=2, space="PSUM") as t0_pool,
            tc.tile_pool(name="t12pool", bufs=2, space="PSUM") as t12_pool,
            tc.tile_pool(name="small", bufs=16) as sm_pool,
            tc.tile_pool(name="tmp", bufs=8) as tmp_pool,
            tc.tile_pool(name="slab", bufs=2) as slab_pool,
        ):
            kt_sb = singles.tile([66, 3, H, P], bf16)
            nc.sync.dma_start(out=kt_sb, in_=kt_d.rearrange("s h d p -> d s h p"))
            vp_sb = singles.tile([128, 3, H, 65], bf16)
            nc.sync.dma_start(out=vp_sb, in_=vp_d.rearrange("s h p c -> p s h c"))

            t_g, t_r1, t_r2 = cfg
            for t in range(N_TILES):
                s0 = t * W_TILE
                Wq = min(W_TILE, SSH - s0)
                nch = Wq // 128
                if t < t_g:
                    cat = "g"
                elif t < t_g + t_r1:
                    cat = "r1"
                elif t < t_g + t_r1 + t_r2:
                    cat = "r2"
                else:
                    cat = "both"

                qt_t = qt_pool.tile([66, H, W_TILE], bf16)
                nc.sync.dma_start(
                    out=qt_t[:, :, :Wq],
                    in_=qt_d.rearrange("h d s -> d h s")[:, :, s0:s0 + Wq])
                if cat == "both":
                    am_t = am_pool.tile([128, 4], f32)
                    nc.sync.dma_start(out=am_t, in_=am_d[t])

                slab = slab_pool.tile([128, 4, H * D], f32)

                for h in range(H):
                    if cat == "g":
                        # segment-0 only: regional == base here
                        sc = sc_pool.tile([128, W_TILE], f32, tag="scores")
                        nc.tensor.matmul(
                            sc[:, :Wq], lhsT=kt_sb[:, 0, h, :],
                            rhs=qt_t[:, h, :Wq], start=True, stop=True)
                        e = e_pool.tile([128, W_TILE], bf16, tag="e")
                        nc.scalar.activation(e[:, :Wq], sc[:, :Wq], Exp)
                        T0 = t0_pool.tile([128, 4, 65], f32, tag="T0")
                        for c in range(nch):
                            cs = slice(c * 128, (c + 1) * 128)
                            nc.tensor.matmul(T0[:, c, :], lhsT=e[:, cs],
                                             rhs=vp_sb[:, 0, h, :],
                                             start=True, stop=True)
                        # out = T0n / (0.5 * T0d)
                        hd = sm_pool.tile([128, 4], f32, tag="sm")
                        nc.vector.tensor_scalar_mul(hd[:, :nch],
                                                    T0[:, :nch, 64], 0.5)
                        rg = sm_pool.tile([128, 4], f32, tag="sm")
                        nc.vector.reciprocal(rg[:, :nch], hd[:, :nch])
                        nc.vector.tensor_mul(
                            slab[:, :nch, h * 64:(h + 1) * 64],
                            T0[:, :nch, 0:64],
                            rg[:, :nch, None].broadcast_to([128, nch, 64]))
                        continue

                    if cat in ("r1", "r2"):
                        # segments {0, r}: regional = pure seg-r softmax,
                        # base = seg-0; no masks needed anywhere
                        rseg = 1 if cat == "r1" else 2
                        es2 = []
                        for r in (0, rseg):
                            sc = sc_pool.tile([128, W_TILE], f32,
                                              tag="scores")
                            nc.tensor.matmul(
                                sc[:, :Wq], lhsT=kt_sb[:, r, h, :],
                                rhs=qt_t[:, h, :Wq], start=True, stop=True)
                            e = e_pool.tile([128, W_TILE], bf16, tag="e")
                            nc.scalar.activation(e[:, :Wq], sc[:, :Wq], Exp)
                            es2.append(e)
                        T0 = t0_pool.tile([128, 4, 65], f32, tag="T0")
                        T1 = t12_pool.tile([128, 4, 65], f32, tag="T12")
                        for c in range(nch):
                            cs = slice(c * 128, (c + 1) * 128)
                            nc.tensor.matmul(T0[:, c, :], lhsT=es2[0][:, cs],
                                             rhs=vp_sb[:, 0, h, :],
                                             start=True, stop=True)
                            nc.tensor.matmul(T1[:, c, :], lhsT=es2[1][:, cs],
                                             rhs=vp_sb[:, rseg, h, :],
                                             start=True, stop=True)
                        rb = sm_pool.tile([128, 4], f32, tag="sm")
                        nc.vector.reciprocal(rb[:, :nch], T0[:, :nch, 64])
                        r1p = sm_pool.tile([128, 4], f32, tag="sm")
                        nc.vector.reciprocal(r1p[:, :nch], T1[:, :nch, 64])
                        t1 = tmp_pool.tile([128, 4, 64], f32, tag="t1")
                        nc.vector.tensor_mul(
                            t1[:, :nch, :], T0[:, :nch, 0:64],
                            rb[:, :nch, None].broadcast_to([128, nch, 64]))
                        t2 = tmp_pool.tile([128, 4, 64], f32, tag="t2")
                        nc.vector.tensor_mul(
                            t2[:, :nch, :], T1[:, :nch, 0:64],
                            r1p[:, :nch, None].broadcast_to([128, nch, 64]))
                        nc.gpsimd.tensor_add(
                            slab[:, :nch, h * 64:(h + 1) * 64],
                            t1[:, :nch, :], t2[:, :nch, :])
                        continue

                    es = []
                    for r in range(3):
                        sc = sc_pool.tile([128, W_TILE], f32, tag="scores")
                        nc.tensor.matmul(
                            sc[:, :Wq],
                            lhsT=kt_sb[:, r, h, :],
                            rhs=qt_t[:, h, :Wq],
                            start=True, stop=True)
                        e = e_pool.tile([128, W_TILE], bf16, tag="e")
                        nc.scalar.activation(e[:, :Wq], sc[:, :Wq], Exp)
                        es.append(e)

                    T0 = t0_pool.tile([128, 4, 65], f32, tag="T0")
                    T12 = t12_pool.tile([128, 4, 65], f32, tag="T12")
                    for c in range(nch):
                        cs = slice(c * 128, (c + 1) * 128)
                        nc.tensor.matmul(T0[:, c, :], lhsT=es[0][:, cs],
                                         rhs=vp_sb[:, 0, h, :],
                                         start=True, stop=True)
                        nc.tensor.matmul(T12[:, c, :], lhsT=es[1][:, cs],
                                         rhs=vp_sb[:, 1, h, :],
                                         start=True, stop=False)
                        nc.tensor.matmul(T12[:, c, :], lhsT=es[2][:, cs],
                                         rhs=vp_sb[:, 2, h, :],
                                         start=False, stop=True)

                    a0 = am_t[:, 0:nch]
                    # wd = a0*T0d + T12d ; rr = 1/wd ; rb = 1/T0d
                    # out = (a0*rr + rb)*T0n + rr*T12n
                    m0 = sm_pool.tile([128, 4], f32, tag="sm")
                    nc.vector.tensor_mul(m0[:, :nch], a0, T0[:, :nch, 64])
                    wd = sm_pool.tile([128, 4], f32, tag="sm")
                    nc.vector.tensor_add(wd[:, :nch], m0[:, :nch],
                                         T12[:, :nch, 64])
                    rb = sm_pool.tile([128, 4], f32, tag="sm")
                    nc.vector.reciprocal(rb[:, :nch], T0[:, :nch, 64])
                    rr = sm_pool.tile([128, 4], f32, tag="sm")
                    nc.vector.reciprocal(rr[:, :nch], wd[:, :nch])
                    c0a = sm_pool.tile([128, 4], f32, tag="sm")
                    nc.gpsimd.tensor_mul(c0a[:, :nch], a0, rr[:, :nch])
                    c0b = sm_pool.tile([128, 4], f32, tag="sm")
                    nc.gpsimd.tensor_add(c0b[:, :nch], c0a[:, :nch],
                                         rb[:, :nch])

                    t1 = tmp_pool.tile([128, 4, 64], f32, tag="t1")
                    nc.vector.tensor_mul(
                        t1[:, :nch, :], T0[:, :nch, 0:64],
                        c0b[:, :nch, None].broadcast_to([128, nch, 64]))
                    t2 = tmp_pool.tile([128, 4, 64], f32, tag="t2")
                    nc.vector.tensor_mul(
                        t2[:, :nch, :], T12[:, :nch, 0:64],
                        rr[:, :nch, None].broadcast_to([128, nch, 64]))
                    nc.gpsimd.tensor_add(
                        slab[:, :nch, h * 64:(h + 1) * 64],
                        t1[:, :nch, :], t2[:, :nch, :])

                nc.sync.dma_start(
                    out=out_d[s0:s0 + Wq, :].rearrange("(c p) f -> p c f", p=128),
                    in_=slab[:, :nch, :])

    nc.compile()
    return nc


def _get_compiled(gt):
    if gt not in _COMPILED:
        _COMPILED[gt] = _build_kernel(gt)
    return _COMPILED[gt]


# ---------------------------------------------------------------- frontend
def _prepare(q, k, v, regional_k, regional_v, region_masks):
    bf = ml_dtypes.bfloat16
    q = np.asarray(q, dtype=np.float32)
    k = np.asarray(k, dtype=np.float32)
    v = np.asarray(v, dtype=np.float32)
    regional_k = np.asarray(regional_k, dtype=np.float32)
    regional_v = np.asarray(regional_v, dtype=np.float32)
    region_masks = np.asarray(region_masks, dtype=np.float32)

    a0, a1, a2 = _preprocess_masks(region_masks)  # [S] each

    # 4-way category sort: global-only / region-1-only / region-2-only /
    # both-regions. Each core gets identical leading tile counts per
    # category (SPMD requires one graph); leftovers fall back to the
    # general "both" path, which is correct for any query.
    cats = [
        np.nonzero(a0 == 1.0)[0],
        np.nonzero((a1 == 1.0) & (a2 == 0.0))[0],
        np.nonzero((a2 == 1.0) & (a1 == 0.0))[0],
    ]
    counts = []
    used_parts = []
    leftover_parts = []
    budget = N_TILES - 1  # keep at least one general tile (incl. ragged tail)
    for idx in cats:
        tcnt = min(len(idx) // (N_CORES * W_TILE), budget)
        budget -= tcnt
        counts.append(tcnt)
        n_used = tcnt * W_TILE * N_CORES
        used_parts.append(idx[:n_used])
        leftover_parts.append(idx[n_used:])
    t_g, t_r1, t_r2 = counts
    leftover_parts.append(np.nonzero((a1 == 1.0) & (a2 == 1.0))[0])
    leftover = np.concatenate(leftover_parts)
    ns = [t_g * W_TILE, t_r1 * W_TILE, t_r2 * W_TILE]
    n_left = SSH - sum(ns)
    perm = np.empty(S, dtype=np.int64)
    for c in range(N_CORES):
        lo = c * SSH
        off = 0
        for ncat, part in zip(ns, used_parts):
            perm[lo + off:lo + off + ncat] = part[c * ncat:(c + 1) * ncat]
            off += ncat
        perm[lo + off:lo + SSH] = leftover[c * n_left:(c + 1) * n_left]
    gt = (t_g, t_r1, t_r2)

    a0p = a0[perm]
    b1 = (NEG_BIAS * (1.0 - a1[perm])).astype(bf)
    b2 = (NEG_BIAS * (1.0 - a2[perm])).astype(bf)

    # qT plus bias rows: [H, 66, S] bf16, query-permuted
    qt = np.empty((H, 66, S), dtype=bf)
    qt[:, :64, :] = q[0].transpose(1, 2, 0)[:, :, perm].astype(bf)
    qt[:, 64, :] = b1[None, :]
    qt[:, 65, :] = b2[None, :]

    # kT*scale plus selector rows: [3, H, 66, P] bf16
    k_segs = np.stack([k[0], regional_k[0, 0], regional_k[1, 0]], axis=0)
    kt = np.zeros((3, H, 66, P), dtype=np.float32)
    kt[:, :, :64, :] = k_segs.transpose(0, 2, 3, 1) * np.float32(SCALE)
    kt[1, :, 64, :] = 1.0
    kt[2, :, 65, :] = 1.0
    kt = kt.astype(bf)

    # V plus 2.0-column: [3, H, P, 65] bf16
    v_segs = np.stack([v[0], regional_v[0, 0], regional_v[1, 0]], axis=0)
    vp = np.empty((3, H, P, 65), dtype=np.float32)
    vp[..., :64] = v_segs.transpose(0, 2, 1, 3)
    vp[..., 64] = 2.0
    vp = vp.astype(bf)

    in_maps = []
    for core in range(N_CORES):
        lo = core * SSH
        am = np.zeros((N_TILES, 128, 4), np.float32)
        for t in range(sum(gt), N_TILES):
            s0 = t * W_TILE
            Wq = min(W_TILE, SSH - s0)
            nch = Wq // 128
            am[t, :, :nch] = a0p[lo + s0: lo + s0 + Wq].reshape(nch, 128).T
        in_maps.append({
            "qt": np.ascontiguousarray(qt[:, :, lo:lo + SSH]),
            "kt": kt,
            "vp": vp,
            "am": am,
        })
    return in_maps, perm, gt


def kernel(q, k, v, regional_k, regional_v, region_masks):
    from concourse.bass_utils import run_bass_kernel_spmd

    in_maps, perm, gt = _prepare(q, k, v, regional_k, regional_v,
                                 region_masks)
    nc = _get_compiled(gt)
    res = run_bass_kernel_spmd(nc, in_maps, core_ids=list(range(N_CORES)))
    out_sorted = np.concatenate(
        [res.results[i]["out"] for i in range(N_CORES)], axis=0)
    out = np.empty_like(out_sorted)
    out[perm] = out_sorted
    return out.reshape(1, S, H * D).astype(np.float32)
